# revision 2
# baseline (speedup 1.0000x reference)
"""Trainium2 Bass kernel v2 for nn_Net_66408784331557 (dense MLP, sync-BN).

Single HBM pass over x, pipelined across 4 feature-groups of 512:
  per group g: DMA x[:, group] (host pre-laid group-major, 32KB descs)
    -> PE fp32 transposes -> fused PSUM->SBUF copy+sum (DVE ttr / ACT accum)
       + square+sumsq (ACT / DVE) -> per-group AllReduce of [sum|sumsq]
       (overlapped with group g+1's DMA) -> exact s-fold into W_g
    -> P += x_g @ (W_g * s_g)^T  (f32r matmuls, accumulated in SBUF)
  h1 = relu(P + c), c = sum_g (beta_g - mu_g*s_g) @ W_g^T + b_in
12 middle layers: f32r matmul, ACT relu+bias+sum-accum, DVE sumsq,
one AllReduce each; head matmul + unpermute at the final activation.

Batch columns are permuted within each 2048-row tile (col = 128*(b%16*?) ...
precisely: within tile, col r*128+p <-> batch 16p+4R+r'); all middle layers
are column-order agnostic; the head activation un-permutes via its out AP.
"""

import sys
import functools

import numpy as np

for _p in ("/opt/trn_rl_repo",):
    if _p not in sys.path:
        sys.path.insert(0, _p)

N_CORES = 8
B = 65536
D = 2048
H = 75
L = 12
N_OUT = 1
EPS = 1e-5

B_LOC = B // N_CORES          # 8192
NG = 8                        # feature groups
GF = D // NG                  # 256 features per group
GQ = GF // 128                # 2 feature chunks of 128 per group
NTILE = B_LOC // 2048         # 4 x-tiles of 2048 rows per group
NCH = B_LOC // 512            # 16 batch chunks of 512


def build_program(debug=False):
    import concourse.bass as bass
    import concourse.mybir as mybir
    import concourse.tile as tile
    from concourse import bacc

    f32 = mybir.dt.float32
    f32r = mybir.dt.float32r
    AF = mybir.ActivationFunctionType
    ALU = mybir.AluOpType

    INV_B = 1.0 / float(B)
    INV_C = 1.0 / float(N_CORES)

    nc = bacc.Bacc("TRN2", target_bir_lowering=False, debug=debug,
                   enable_asserts=True, num_devices=N_CORES)

    # ---- I/O ----
    xg_d = nc.dram_tensor("xg", [NG, B_LOC, GF], f32, kind="ExternalInput").ap()
    wint_d = nc.dram_tensor("wint", [128, NG * GQ, H], f32, kind="ExternalInput").ap()
    bin_d = nc.dram_tensor("bin", [H, 1], f32, kind="ExternalInput").ap()
    grow_d = nc.dram_tensor("grow", [128, NG * GQ], f32, kind="ExternalInput").ap()
    brow_d = nc.dram_tensor("brow", [128, NG * GQ], f32, kind="ExternalInput").ap()
    midwt_d = nc.dram_tensor("midwt", [H, L, H], f32, kind="ExternalInput").ap()
    midg_d = nc.dram_tensor("midg", [H, L], f32, kind="ExternalInput").ap()
    midbeta_d = nc.dram_tensor("midbeta", [H, L], f32, kind="ExternalInput").ap()
    midbias_d = nc.dram_tensor("midbias", [H, L], f32, kind="ExternalInput").ap()
    woutt_d = nc.dram_tensor("woutt", [H, N_OUT], f32, kind="ExternalInput").ap()
    bout_d = nc.dram_tensor("bout", [1, 1], f32, kind="ExternalInput").ap()
    identf_d = nc.dram_tensor("identf", [128, 128], f32, kind="ExternalInput").ap()
    out_d = nc.dram_tensor("out", [B_LOC, N_OUT], f32, kind="ExternalOutput").ap()


    rg = [list(range(N_CORES))]
    # x tile view: [g, t, p, r, f] ; batch row = t*2048 + p*16 + r
    xv = xg_d.rearrange("g (t p r) f -> g t p r f", p=128, r=16)

    with tile.TileContext(nc) as tc:
        with tc.tile_pool(name="const", bufs=1) as cp, \
             tc.tile_pool(name="drp", bufs=1, space="DRAM") as drp:

            # ---- constants ----
            wint_sb = cp.tile([128, NG * GQ, H], f32)
            nc.sync.dma_start(wint_sb, wint_d)
            bin_sb = cp.tile([H, 1], f32)
            nc.sync.dma_start(bin_sb, bin_d)
            grow_sb = cp.tile([128, NG * GQ], f32)
            nc.sync.dma_start(grow_sb, grow_d)
            brow_sb = cp.tile([128, NG * GQ], f32)
            nc.sync.dma_start(brow_sb, brow_d)
            midwt_sb = cp.tile([H, L, H], f32)
            nc.sync.dma_start(midwt_sb, midwt_d)
            midg_sb = cp.tile([H, L], f32)
            nc.sync.dma_start(midg_sb, midg_d)
            midbeta_sb = cp.tile([H, L], f32)
            nc.sync.dma_start(midbeta_sb, midbeta_d)
            midbias_sb = cp.tile([H, L], f32)
            nc.sync.dma_start(midbias_sb, midbias_d)
            woutt_sb = cp.tile([H, N_OUT], f32)
            nc.sync.dma_start(woutt_sb, woutt_d)
            bout_sb = cp.tile([1, 1], f32)
            nc.sync.dma_start(bout_sb, bout_d)
            identf = cp.tile([128, 128], f32)
            nc.sync.dma_start(identf, identf_d)

            epsc = cp.tile([128, 1], f32)
            nc.vector.memset(epsc, float(EPS))

            # ---- dummy collective at t=0 to absorb first-AR warmup ----
            dmy_i = drp.tile([1, 64], f32, name="dmy_i")
            dmy_o = drp.tile([1, 64], f32, name="dmy_o")
            dmyrow = cp.tile([1, 64], f32)
            nc.vector.memset(dmyrow, 0.0)
            nc.scalar.dma_start(dmy_i, dmyrow)
            nc.gpsimd.collective_compute(
                "AllReduce", mybir.AluOpType.add, replica_groups=rg,
                ins=[dmy_i.opt()], outs=[dmy_o.opt()])

            # ---- persistent buffers ----
            hp = tc.alloc_tile_pool(name="hpool", bufs=1)
            h_a = hp.tile([H, B_LOC], f32r)  # P accumulator, then h1 (in-place)

            # stats partials / payload staging
            stp = tc.alloc_tile_pool(name="stats", bufs=1)
            pays = [stp.tile([128, 2 * GQ], f32, name=f"payg{g}") for g in range(NG)]
            g2s = [stp.tile([128, 2 * GQ], f32, name=f"g2g{g}") for g in range(NG)]
            # mid-layer payloads [128,2] (rows >= H unused but zeroed once)
            mpays = [stp.tile([128, 2], f32, name=f"mpay{l}") for l in range(L + 1)]
            mg2s = [stp.tile([128, 2], f32, name=f"mg2{l}") for l in range(L + 1)]
            for l in range(L + 1):
                nc.vector.memset(mpays[l], 0.0)
            BNST = stp.tile([H, NCH, 6], f32)

            # folded input weights per group
            wfp = tc.alloc_tile_pool(name="wf", bufs=1)
            wfold = wfp.tile([128, NG * GQ, H], f32r)

            # ========== PASS A: single sweep over x ==========
            with tc.tile_pool(name="xin", bufs=2) as xp, \
                 tc.tile_pool(name="xts", bufs=40) as xtp, \
                 tc.tile_pool(name="sqs", bufs=4) as sqp, \
                 tc.tile_pool(name="pst", bufs=4, space="PSUM") as pstp, \
                 tc.tile_pool(name="psh", bufs=2, space="PSUM") as pshp, \
                 tc.tile_pool(name="psc", bufs=1, space="PSUM") as pscp:

                ps_c = pscp.tile([H, 1], f32)

                xts_all = {}

                def emit_P(g, t):
                    # P matmuls for (group g, tile t); requires wfold_g folded
                    for R in range(4):
                        idx = t * 4 + R
                        psh = pshp.tile([H, 512], f32, tag="psh",
                                        name=f"psh{g}_{t}_{R}")
                        for q in range(GQ):
                            nc.tensor.matmul(
                                psh, wfold[:, g * GQ + q, :],
                                xts_all.pop((g, t, R, q)),
                                start=(q == 0), stop=(q == GQ - 1),
                                skip_group_check=True)
                        sl = slice(idx * 512, (idx + 1) * 512)
                        if g == 0:
                            nc.scalar.copy(h_a[:, sl], psh)
                        else:
                            nc.vector.tensor_tensor(
                                out=h_a[:, sl], in0=h_a.bitcast(f32)[:, sl],
                                in1=psh, op=ALU.add)

                for g in range(NG):
                    BN = stp.tile([128, GQ, NTILE * 4, 6], f32, tag="bnstp",
                                  bufs=2, name=f"bnstp{g}")
                    for t in range(NTILE):
                        xt = xp.tile([128, 16, GF], f32, tag="xt", name=f"x{g}_{t}")
                        nc.sync.dma_start(xt, xv[g, t])
                        for R in range(4):  # r-quads -> one 512-batch chunk each
                            idx = t * 4 + R
                            for q in range(GQ):
                                pst = pstp.tile([128, 512], f32, tag="pst",
                                                name=f"pst{g}_{t}_{R}_{q}")
                                for rp in range(4):
                                    r = 4 * R + rp
                                    nc.tensor.matmul(
                                        pst[:, rp * 128:(rp + 1) * 128],
                                        xt[:, r, q * 128:(q + 1) * 128],
                                        identf, is_transpose=True,
                                        skip_group_check=True)
                                xts = xtp.tile([128, 512], f32r, tag="xts",
                                               name=f"xts{g}_{t}_{R}_{q}")
                                nc.scalar.copy(xts, pst)
                                nc.vector.bn_stats(BN[:, q, idx, :],
                                                   xts.bitcast(f32))
                                xts_all[(g, t, R, q)] = xts
                        if g >= 1:
                            emit_P(g - 1, t)

                    # ---- group stats -> AllReduce ----
                    pay = pays[g]
                    for q in range(GQ):
                        mvg = stp.tile([128, 2], f32, tag="mvg",
                                       name=f"mvg{g}_{q}", bufs=2)
                        nc.vector.bn_aggr(mvg, BN[:, q])
                        nc.vector.tensor_scalar_mul(pay[:, q:q + 1],
                                                    mvg[:, 0:1], INV_C)
                        m2g = stp.tile([128, 1], f32, tag="m2g",
                                       name=f"m2g{g}_{q}", bufs=2)
                        nc.vector.tensor_tensor(out=m2g, in0=mvg[:, 0:1],
                                                in1=mvg[:, 0:1], op=ALU.mult)
                        nc.vector.tensor_tensor(out=m2g, in0=mvg[:, 1:2],
                                                in1=m2g, op=ALU.add)
                        nc.vector.tensor_scalar_mul(pay[:, GQ + q:GQ + q + 1],
                                                    m2g, INV_C)
                    sti = drp.tile([128, 2 * GQ], f32, name=f"sti{g}")
                    sto = drp.tile([128, 2 * GQ], f32, name=f"sto{g}")
                    nc.scalar.dma_start(sti, pay)
                    nc.gpsimd.collective_compute(
                        "AllReduce", mybir.AluOpType.add, replica_groups=rg,
                        ins=[sti.opt()], outs=[sto.opt()])
                    g2 = g2s[g]
                    nc.scalar.dma_start(g2, sto)

                    # ---- fold math in [128, GQ] layout ----
                    mu = g2[:, 0:GQ]
                    ex2 = g2[:, GQ:2 * GQ]
                    musq = stp.tile([128, GQ], f32, tag="musq", name=f"musq{g}", bufs=2)
                    nc.vector.tensor_tensor(out=musq, in0=mu, in1=mu, op=ALU.mult)
                    vef = stp.tile([128, GQ], f32, tag="vef", name=f"vef{g}", bufs=2)
                    nc.vector.tensor_tensor(out=vef, in0=ex2, in1=musq, op=ALU.subtract)
                    sd = stp.tile([128, GQ], f32, tag="sd", name=f"sd{g}", bufs=2)
                    nc.scalar.activation(sd, vef, AF.Sqrt, bias=epsc[:, 0:1])
                    r0 = stp.tile([128, GQ], f32, tag="r0", name=f"r0{g}", bufs=2)
                    nc.vector.reciprocal(r0, sd)
                    # Newton polish: r = r0*(1.5 - 0.5*(vef+eps)*r0^2)
                    t1 = stp.tile([128, GQ], f32, tag="t1", name=f"t1{g}", bufs=2)
                    nc.vector.tensor_tensor(out=t1, in0=r0, in1=r0, op=ALU.mult)
                    vep = stp.tile([128, GQ], f32, tag="vep", name=f"vep{g}", bufs=2)
                    nc.vector.tensor_scalar(out=vep, in0=vef, scalar1=1.0,
                                            scalar2=float(EPS), op0=ALU.mult,
                                            op1=ALU.add)
                    nc.vector.tensor_tensor(out=t1, in0=vep, in1=t1, op=ALU.mult)
                    nc.vector.tensor_scalar(out=t1, in0=t1, scalar1=-0.5, scalar2=1.5,
                                            op0=ALU.mult, op1=ALU.add)
                    rr = stp.tile([128, GQ], f32, tag="rr", name=f"rr{g}", bufs=2)
                    nc.vector.tensor_tensor(out=rr, in0=r0, in1=t1, op=ALU.mult)
                    srow = stp.tile([128, GQ], f32, tag="srow", name=f"srow{g}", bufs=2)
                    nc.vector.tensor_tensor(out=srow, in0=rr,
                                            in1=grow_sb[:, g * GQ:(g + 1) * GQ],
                                            op=ALU.mult)
                    mt = stp.tile([128, GQ], f32, tag="mt", name=f"mt{g}", bufs=2)
                    nc.vector.tensor_tensor(out=mt, in0=mu, in1=srow, op=ALU.mult)
                    trow = stp.tile([128, GQ], f32, tag="trow", name=f"trow{g}", bufs=2)
                    nc.vector.tensor_tensor(out=trow,
                                            in0=brow_sb[:, g * GQ:(g + 1) * GQ],
                                            in1=mt, op=ALU.subtract)
                    # fold into W
                    for q in range(GQ):
                        nc.vector.tensor_scalar_mul(
                            wfold[:, g * GQ + q, :], wint_sb[:, g * GQ + q, :],
                            srow[:, q:q + 1])
                    # c partial: ps_c += W_g^T-chunks @ t_q
                    for q in range(GQ):
                        nc.tensor.matmul(ps_c, wint_sb[:, g * GQ + q, :],
                                         trow[:, q:q + 1],
                                         start=(g == 0 and q == 0),
                                         stop=(g == NG - 1 and q == GQ - 1),
                                         skip_group_check=True)

                for t in range(NTILE):
                    emit_P(NG - 1, t)

                # c = ps_c + b_in
                cvec = stp.tile([H, 1], f32)
                nc.vector.tensor_tensor(out=cvec, in0=ps_c, in1=bin_sb, op=ALU.add)
            wfp.release()

            hp2 = tc.alloc_tile_pool(name="hpool2", bufs=1)
            h_b = hp2.tile([H, B_LOC], f32r)

            # ========== h1 = relu(P + c), with stats ==========
            with tc.tile_pool(name="l1ps", bufs=3, space="PSUM") as mps, \
                 tc.tile_pool(name="l1pb", bufs=2, space="PSUM") as mpb, \
                 tc.tile_pool(name="mid", bufs=2) as mp_:
                for ch in range(NCH):
                    sl = slice(ch * 512, (ch + 1) * 512)
                    nc.scalar.activation(h_a[:, sl], h_a.bitcast(f32)[:, sl],
                                         AF.Relu, bias=cvec[:, 0:1])
                    nc.vector.bn_stats(BNST[:, ch, :], h_a.bitcast(f32)[:, sl])

                # ========== 12 middle layers ==========
                h_in, h_out = h_a, h_b
                for l in range(L):
                    pay = mpays[l]
                    mv = mp_.tile([H, 2], f32, tag="mv", name=f"mv{l}")
                    nc.vector.bn_aggr(mv, BNST)
                    # pay = [mean/ncores, (var+mean^2)/ncores]
                    msqp = mp_.tile([H, 1], f32, tag="msqp", name=f"msqp{l}")
                    nc.vector.tensor_tensor(out=msqp, in0=mv[:, 0:1],
                                            in1=mv[:, 0:1], op=ALU.mult)
                    nc.vector.tensor_scalar_mul(pay[0:H, 0:1], mv[:, 0:1], INV_C)
                    nc.vector.tensor_tensor(out=msqp, in0=mv[:, 1:2], in1=msqp,
                                            op=ALU.add)
                    nc.vector.tensor_scalar_mul(pay[0:H, 1:2], msqp, INV_C)
                    mbi = drp.tile([128, 2], f32, name=f"mbi{l}")
                    mbo = drp.tile([128, 2], f32, name=f"mbo{l}")
                    nc.scalar.dma_start(mbi, pay)
                    nc.gpsimd.collective_compute(
                        "AllReduce", mybir.AluOpType.add, replica_groups=rg,
                        ins=[mbi.opt()], outs=[mbo.opt()])
                    g2 = mg2s[l]
                    nc.scalar.dma_start(g2, mbo)

                    mug = mp_.tile([H, 1], f32, tag="mug", name=f"mug{l}")
                    nc.vector.tensor_copy(mug, g2[0:H, 0:1])
                    ex2 = mp_.tile([H, 1], f32, tag="mex2", name=f"mex2{l}")
                    nc.vector.tensor_copy(ex2, g2[0:H, 1:2])
                    msq = mp_.tile([H, 1], f32, tag="msq", name=f"msq{l}")
                    nc.vector.tensor_tensor(out=msq, in0=mug, in1=mug, op=ALU.mult)
                    vef = mp_.tile([H, 1], f32, tag="mvef", name=f"mvef{l}")
                    nc.vector.tensor_tensor(out=vef, in0=ex2, in1=msq,
                                            op=ALU.subtract)
                    sd = mp_.tile([H, 1], f32, tag="msd", name=f"msd{l}")
                    nc.scalar.activation(sd, vef, AF.Sqrt, bias=epsc[0:H, 0:1])
                    r0 = mp_.tile([H, 1], f32, tag="mr0", name=f"mr0{l}")
                    nc.vector.reciprocal(r0, sd)
                    t1 = mp_.tile([H, 1], f32, tag="mt1", name=f"mt1{l}")
                    nc.vector.tensor_tensor(out=t1, in0=r0, in1=r0, op=ALU.mult)
                    vep = mp_.tile([H, 1], f32, tag="mvep", name=f"mvep{l}")
                    nc.vector.tensor_scalar(out=vep, in0=vef, scalar1=1.0,
                                            scalar2=float(EPS), op0=ALU.mult,
                                            op1=ALU.add)
                    nc.vector.tensor_tensor(out=t1, in0=vep, in1=t1, op=ALU.mult)
                    nc.vector.tensor_scalar(out=t1, in0=t1, scalar1=-0.5,
                                            scalar2=1.5, op0=ALU.mult, op1=ALU.add)
                    rr = mp_.tile([H, 1], f32, tag="mrr", name=f"mrr{l}")
                    nc.vector.tensor_tensor(out=rr, in0=r0, in1=t1, op=ALU.mult)
                    s2 = mp_.tile([H, 1], f32, tag="ms2", name=f"ms2{l}")
                    nc.vector.tensor_tensor(out=s2, in0=rr, in1=midg_sb[:, l:l + 1],
                                            op=ALU.mult)
                    mt = mp_.tile([H, 1], f32, tag="mmt", name=f"mmt{l}")
                    nc.vector.tensor_tensor(out=mt, in0=mug, in1=s2, op=ALU.mult)
                    t2 = mp_.tile([H, 1], f32, tag="mt2", name=f"mt2{l}")
                    nc.vector.tensor_tensor(out=t2, in0=midbeta_sb[:, l:l + 1],
                                            in1=mt, op=ALU.subtract)
                    wf = mp_.tile([H, H], f32r, tag="wf", name=f"wf{l}")
                    nc.vector.tensor_scalar_mul(wf, midwt_sb[:, l, :], s2)
                    ps_b2 = mpb.tile([H, 1], f32, tag="psb2", name=f"psb2_{l}")
                    nc.tensor.matmul(ps_b2, midwt_sb[:, l, :], t2,
                                     skip_group_check=True)
                    bias2 = mp_.tile([H, 1], f32, tag="bias2", name=f"bias2{l}")
                    nc.vector.tensor_tensor(out=bias2, in0=ps_b2,
                                            in1=midbias_sb[:, l:l + 1], op=ALU.add)

                    for ch in range(NCH):
                        sl = slice(ch * 512, (ch + 1) * 512)
                        psm = mps.tile([H, 512], f32, tag="psm", name=f"psm{l}_{ch}")
                        nc.tensor.matmul(psm, wf, h_in[:, sl],
                                         skip_group_check=True)
                        nc.scalar.activation(h_out[:, sl], psm, AF.Relu,
                                             bias=bias2[:, 0:1])
                        nc.vector.bn_stats(BNST[:, ch, :], h_out.bitcast(f32)[:, sl])
                    h_in, h_out = h_out, h_in

                # ========== head ==========
                woutt_r = mp_.tile([H, N_OUT], f32r, bufs=1)
                nc.vector.tensor_copy(woutt_r, woutt_sb)
                out_row = mp_.tile([1, B_LOC], f32, bufs=1)
                for ch in range(NCH):
                    sl = slice(ch * 512, (ch + 1) * 512)
                    pso = mps.tile([1, 512], f32, tag="pso", name=f"pso{ch}")
                    nc.tensor.matmul(pso, woutt_r, h_in[:, sl],
                                     skip_group_check=True)
                    # unpermute: chunk ch = (t, R); batch = 2048 t + 16 p + (4R + r')
                    t, R = divmod(ch, 4)
                    nc.scalar.activation(
                        out_row.rearrange("o (t p rp) -> o t rp p", t=NTILE, p=128,
                                          rp=16)[:, t, 4 * R:4 * R + 4, :],
                        pso.rearrange("o (rp p) -> o rp p", rp=4, p=128),
                        AF.Identity, bias=bout_sb[0:1, 0:1])
                nc.sync.dma_start(out_d.rearrange("b o -> o b"), out_row)
            hp2.release()
            stp.release()
            hp.release()

    nc.compile()
    return nc


def make_in_maps(inputs):
    """Host-side prep: shard + group-major relayout of x, weight relayouts."""
    x = np.asarray(inputs["x"], np.float32)
    # W_in.T [2048, 75] -> [128, 16, 75] with col (g*GQ+q) <-> feature 512g+128q+p
    wint = np.ascontiguousarray(
        np.asarray(inputs["W_in"], np.float32).T
        .reshape(NG, GQ, 128, H).transpose(2, 0, 1, 3).reshape(128, NG * GQ, H))
    bin_ = np.asarray(inputs["b_in"], np.float32).reshape(-1, 1)
    grow = np.ascontiguousarray(
        np.asarray(inputs["bn_gamma_in"], np.float32)
        .reshape(NG, GQ, 128).transpose(2, 0, 1).reshape(128, NG * GQ))
    brow = np.ascontiguousarray(
        np.asarray(inputs["bn_beta_in"], np.float32)
        .reshape(NG, GQ, 128).transpose(2, 0, 1).reshape(128, NG * GQ))
    midwt = np.ascontiguousarray(
        np.asarray(inputs["mid_W"], np.float32).transpose(2, 0, 1))  # [75, 12, 75]
    midg = np.ascontiguousarray(np.asarray(inputs["mid_gamma"], np.float32).T)
    midbeta = np.ascontiguousarray(np.asarray(inputs["mid_beta"], np.float32).T)
    midbias = np.ascontiguousarray(np.asarray(inputs["mid_b"], np.float32).T)
    woutt = np.ascontiguousarray(np.asarray(inputs["W_out"], np.float32).T)
    bout = np.asarray(inputs["b_out"], np.float32).reshape(1, 1)
    identf = np.eye(128, dtype=np.float32)

    common = dict(wint=wint, bin=bin_, grow=grow, brow=brow, midwt=midwt,
                  midg=midg, midbeta=midbeta, midbias=midbias, woutt=woutt,
                  bout=bout, identf=identf)
    in_maps = []
    for c in range(N_CORES):
        xc = x[c * B_LOC:(c + 1) * B_LOC]            # [8192, 2048]
        xgc = np.ascontiguousarray(
            xc.reshape(B_LOC, NG, GF).transpose(1, 0, 2))  # [4, 8192, 512]
        m = dict(common)
        m["xg"] = xgc
        in_maps.append(m)
    return in_maps


@functools.lru_cache(maxsize=1)
def _get_program():
    return build_program()


def kernel(**inputs) -> np.ndarray:
    from concourse.bass_utils import run_bass_kernel_spmd
    nc = _get_program()
    in_maps = make_in_maps(inputs)
    res = run_bass_kernel_spmd(nc, in_maps, core_ids=list(range(N_CORES)))
    out = np.concatenate([res.results[c]["out"] for c in range(N_CORES)], axis=0)
    return out.astype(np.float32)


if __name__ == "__main__":
    nc = build_program()
    print("built ok:", len(nc.inst_map), "instructions")
